# revision 14
# baseline (speedup 1.0000x reference)
"""EnhancedFlowGNN forward pass on 8 Trainium2 NeuronCores (Bass/Tile), v2.

Strategy (edge parallelism aligned with a node partition, no all-reduce):
  - Host sorts edges by destination ("row") and shards them by row range so
    core i owns nodes [i*6250, (i+1)*6250) and every edge targeting them.
  - All per-edge data movement is BULK: per 128-dest-node block, gpsimd
    dma_gather instructions fetch every edge's table row (node table split
    into two <32768-row windows so indices fit int16) and every edge's
    ssrc[row] — 3 gather instructions per block instead of one indirect DMA
    per 128-edge chunk.
  - All per-edge compute is batched per block: one DVE op builds every
    one-hot scatter matrix, one (or five) DVE ops apply the softmax weights
    in-place on the gathered rows. Only the PSUM-accumulating scatter
    matmuls remain per-chunk.
  - Softmax denominator arrives through ones-columns in the table; no
    segment_max (|logits| stay O(1) for this model family) and no per-edge
    alpha materialization.
  - BN scale is folded into the value weights host-side; BN shift + value
    bias are folded into the residual streams post-softmax.
  - The gather/softmax/scatter pipeline runs in bf16 (output = x[:,-3:] +
    small delta, so datapath errors land ~1e-4 relative).
"""

import numpy as np

import concourse.bass as bass
import concourse.mybir as mybir
import concourse.tile as tile
from concourse.bass import AP
from concourse.tile import ScopedClock

f32 = mybir.dt.float32
bf16 = mybir.dt.bfloat16
i16 = mybir.dt.int16
i32 = mybir.dt.int32

N = 50000
E = 800000
D_IN = 18
H = 128
HEADS = 4
DH = H // HEADS
D_OUT = 3
NEG = 0.2
BN_EPS = 1e-5

NCORES = 8
NSH = N // NCORES            # 6250 nodes per core
NBLK = (NSH + 127) // 128    # 49 blocks (48 full + one of 106)
NSHP = NBLK * 128            # padded node count (ssrc table rows)
P = 128
SPLIT = 32768                # node-table window split (int16 index limit)

NH = (1, 4, 1)               # heads per attention layer
# gathered row: [vals 0:128 | sdst 128:128+nh | ones .. 128+2nh | .. 256]
# table-build matmul also emits ssrc at cols 136:136+nh


# ---------------------------------------------------------------------------
# container compat patches (older walrus in this image)
# ---------------------------------------------------------------------------

_patched = False


def _apply_patches():
    global _patched
    if _patched:
        return
    _patched = True

    from concourse.bass import compact_to_ranges

    # The walrus here accepts at most ONE sync-wait command per instruction,
    # and the EVSEM range-clear in the Tile tail lowers to an InstISA
    # encoding it rejects. Each kernel() call builds + loads a fresh NEFF,
    # so semaphores start zeroed and the tail clears can be dropped.
    def _drain_and_barrier(self, tick_clock, wait_clock):
        nc = self.nc
        drain_inst = nc.sync.drain()
        wait_clock.add_sem_waits(
            drain_inst.ins, ScopedClock({None: tick_clock.global_clock})
        )
        nc.all_engine_barrier()
        popped = nc._tile_sem_poison_stack.pop()
        assert popped is self._sem_poison
        sems = list(self.sems.allocated().values())
        if sems:
            sem_nums = [
                s.num if isinstance(s, bass.SemaphoreHandle) else s for s in sems
            ]
            for sem_range in compact_to_ranges(sem_nums):
                nc.gpsimd.dma_reset(sem_range)
            nc._state.prepend_free_semaphores(sem_nums)
            for poison_set in nc._tile_sem_poison_stack:
                poison_set.update(sem_nums)
        nc.all_engine_barrier()

    tile.TileContext._drain_and_barrier = _drain_and_barrier


_WAITSPLIT_CTR = [0]


def _split_multi_waits(nc, max_waits=1):
    """Move extra sync waits onto same-engine NoOps (walrus limit: 1/inst)."""
    for f in nc.m.functions:
        for b in f.blocks:
            insts = b.instructions
            i = 0
            while i < len(insts):
                inst = insts[i]
                si = inst.sync_info
                if si is not None:
                    waits = list(si.on_wait)
                    imm = [w for w in waits if w.wait_reg is None]
                    reg = [w for w in waits if w.wait_reg is not None]
                    budget = max(0, max_waits - len(reg))
                    if len(imm) > budget:
                        keep = imm[len(imm) - budget:] if budget else []
                        extras = imm[: len(imm) - budget]
                        si.on_wait = reg + keep
                        for j in range(0, len(extras), max_waits):
                            _WAITSPLIT_CTR[0] += 1
                            nop = mybir.InstNoOp(
                                name=f"I-waitsplit-{_WAITSPLIT_CTR[0]}"
                            )
                            nop.engine = inst.engine
                            nop.sync_info = mybir.SyncInfo(
                                on_wait=extras[j: j + max_waits], on_update=[]
                            )
                            insts.insert(i, nop)
                            i += 1
                i += 1


# ---------------------------------------------------------------------------
# host-side preprocessing
# ---------------------------------------------------------------------------

def _pack16(arr):
    """Wrap a flat int list into the dma_gather index layout: index i at
    partition i%16, slot i//16, replicated to all 8 Q7-core groups."""
    w = arr.reshape(-1, 16).T.astype(np.int16)      # [16, n/16]
    return np.tile(w, (8, 1))                       # [128, n/16]


def _preprocess(edge_index):
    row = edge_index[0].astype(np.int64)
    col = edge_index[1].astype(np.int64)
    order = np.argsort(row, kind="stable")
    rs, cs = row[order], col[order]

    per_core = []
    SL = np.zeros(NBLK, np.int64)
    SH = np.zeros(NBLK, np.int64)
    for ci in range(NCORES):
        lo = np.searchsorted(rs, ci * NSH, "left")
        hi = np.searchsorted(rs, (ci + 1) * NSH, "left")
        r = rs[lo:hi] - ci * NSH
        c = cs[lo:hi]
        blocks = []
        for b in range(NBLK):
            blo = np.searchsorted(r, b * 128, "left")
            bhi = np.searchsorted(r, min((b + 1) * 128, NSH), "left")
            rr = r[blo:bhi] - b * 128
            cc = c[blo:bhi]
            o2 = np.argsort(cc, kind="stable")   # ascending-address gathers
            rr, cc = rr[o2], cc[o2]
            m = cc < SPLIT
            blocks.append(((rr[m], cc[m]), (rr[~m], cc[~m] - SPLIT)))
            SL[b] = max(SL[b], (int(m.sum()) + 127) // 128)
            SH[b] = max(SH[b], (int((~m).sum()) + 127) // 128)
        per_core.append(blocks)

    SL = np.maximum(SL, 1)
    SH = np.maximum(SH, 1)
    SLt, SHt = int(SL.sum()), int(SH.sum())
    C2 = SLt + SHt

    import ml_dtypes
    rowrel = np.full((NCORES, P, C2), -1.0, np.float32)
    idxa = np.zeros((NCORES, P, SLt * 8), np.int16)
    idxb = np.zeros((NCORES, P, SHt * 8), np.int16)
    rowloc = np.zeros((NCORES, P, C2 * 8), np.int16)
    for ci in range(NCORES):
        kA = kB = k2 = 0
        for b in range(NBLK):
            (rl, cl), (rh, ch) = per_core[ci][b]
            nL, nH = SL[b] * 128, SH[b] * 128
            fl = np.zeros(nL, np.int64)
            fl[:len(cl)] = cl
            fh = np.zeros(nH, np.int64)
            fh[:len(ch)] = ch
            rloc = np.zeros(nL + nH, np.int64)
            rloc[:len(rl)] = b * 128 + rl
            rloc[nL:nL + len(rh)] = b * 128 + rh
            idxa[ci, :, kA * 8:(kA + SL[b]) * 8] = _pack16(fl)
            idxb[ci, :, kB * 8:(kB + SH[b]) * 8] = _pack16(fh)
            rowloc[ci, :, k2 * 8:(k2 + SL[b] + SH[b]) * 8] = _pack16(rloc)
            rr2 = np.full(nL + nH, -1.0, np.float32)
            rr2[:len(rl)] = rl
            rr2[nL:nL + len(rh)] = rh
            rowrel[ci, :, k2:k2 + SL[b] + SH[b]] = rr2.reshape(-1, 128).T
            kA += SL[b]
            kB += SH[b]
            k2 += SL[b] + SH[b]
        assert kA == SLt and kB == SHt and k2 == C2

    return (list(SL), list(SH), SLt, SHt, C2,
            rowrel.astype(ml_dtypes.bfloat16), idxa, idxb, rowloc)


# ---------------------------------------------------------------------------
# device kernel
# ---------------------------------------------------------------------------

def _build(SL, SH, SLt, SHt, C2):
    from concourse import library_config
    from concourse.library_overlay import lower_extended_insts

    nc = bass.Bass("TRN2", target_bir_lowering=False)

    d_rowrel = nc.dram_tensor("rowrel", [P, C2], bf16, kind="ExternalInput")
    d_idxa = nc.dram_tensor("idxa", [P, SLt * 8], i16, kind="ExternalInput")
    d_idxb = nc.dram_tensor("idxb", [P, SHt * 8], i16, kind="ExternalInput")
    d_rowloc = nc.dram_tensor("rowloc", [P, C2 * 8], i16,
                              kind="ExternalInput")
    d_x128 = nc.dram_tensor("x128", [N, 128], bf16, kind="ExternalInput")
    d_xT = nc.dram_tensor("xT", [NBLK, 20, P], f32, kind="ExternalInput")
    d_xb3 = nc.dram_tensor("xb3", [P, NBLK * 3], f32, kind="ExternalInput")
    d_wcat0 = nc.dram_tensor("wcat0", [52, P], f32, kind="ExternalInput")
    d_wv = nc.dram_tensor("wv", [3, P, 140], bf16, kind="ExternalInput")
    d_hvbn = nc.dram_tensor("hvbn", [3, P], f32, kind="ExternalInput")
    d_wo1 = nc.dram_tensor("wo1", [P, P], f32, kind="ExternalInput")
    d_bo1 = nc.dram_tensor("bo1", [1, P], f32, kind="ExternalInput")
    d_wo2 = nc.dram_tensor("wo2", [P, D_OUT], f32, kind="ExternalInput")
    d_bo2 = nc.dram_tensor("bo2", [1, D_OUT], f32, kind="ExternalInput")
    d_out = nc.dram_tensor("out", [NSH, D_OUT], f32, kind="ExternalOutput")

    tloc = [nc.dram_tensor(f"tloc{l}", [NSH, 256], bf16) for l in range(3)]
    tfull = [nc.dram_tensor(f"tfull{l}", [N, 256], bf16,
                            addr_space="Shared") for l in range(3)]
    ssrc_d = [nc.dram_tensor(f"ssrc{l}", [NSHP, 128], bf16) for l in range(3)]
    srow_d = [nc.dram_tensor(f"srow{l}", [NBLK, P], bf16) for l in range(3)]

    AL = mybir.AluOpType
    AF = mybir.ActivationFunctionType

    def blk_valid(b):
        return P if b < NBLK - 1 else NSH - (NBLK - 1) * 128

    with tile.TileContext(nc) as tc:
        with tile_pools(tc) as (res, wk, gp, up, ps, pst):

            # ---- constants / resident tiles (gpsimd iota BEFORE the
            # library reload; no gpsimd compute afterwards) ----
            # explicit standard-library load first: a re-execution of this
            # NEFF starts with the mlp library still resident from the
            # previous run, and iota needs standard ucode
            nc.gpsimd.load_library(library_config.standard)
            iota_i = res.tile([P, P], i32)
            nc.gpsimd.iota(iota_i[:], pattern=[[1, P]], base=0,
                           channel_multiplier=0)
            iop_i = res.tile([P, 1], i32)
            nc.gpsimd.iota(iop_i[:], pattern=[[0, 1]], base=0,
                           channel_multiplier=1)
            nc.gpsimd.load_library(library_config.mlp)

            iota_f = res.tile([P, P], f32)
            nc.vector.tensor_copy(iota_f[:], iota_i[:])
            iota_bf = res.tile([P, P], bf16)
            nc.vector.tensor_copy(iota_bf[:], iota_i[:])
            iop_f = res.tile([P, 1], f32)
            nc.vector.tensor_copy(iop_f[:], iop_i[:])
            ident = res.tile([P, P], f32)
            nc.vector.tensor_scalar(out=ident[:], in0=iota_f[:],
                                    scalar1=iop_f[:], scalar2=None,
                                    op0=AL.is_equal)
            alpha_t = res.tile([P, 1], f32)
            nc.vector.memset(alpha_t[:], NEG)

            rowrel_t = res.tile([P, C2], bf16)
            nc.sync.dma_start(out=rowrel_t[:], in_=d_rowrel[:])
            idxa_t = res.tile([P, SLt * 8], i16)
            nc.sync.dma_start(out=idxa_t[:], in_=d_idxa[:])
            idxb_t = res.tile([P, SHt * 8], i16)
            nc.sync.dma_start(out=idxb_t[:], in_=d_idxb[:])
            rowloc_t = res.tile([P, C2 * 8], i16)
            nc.sync.dma_start(out=rowloc_t[:], in_=d_rowloc[:])
            wcat0_t = res.tile([52, P], f32)
            nc.sync.dma_start(out=wcat0_t[:], in_=d_wcat0[:])
            wv_t = []
            for l in range(3):
                wvl = res.tile([P, 140], bf16, tag=f"wv{l}")
                wv_t.append(wvl)
            for l in range(3):
                nc.sync.dma_start(out=wv_t[l][:], in_=d_wv[l, :, :])
            wo1_t = res.tile([P, P], f32)
            nc.sync.dma_start(out=wo1_t[:], in_=d_wo1[:])
            wo2_t = res.tile([P, D_OUT], f32)
            nc.sync.dma_start(out=wo2_t[:], in_=d_wo2[:])
            xb3_t = res.tile([P, NBLK * 3], f32)
            nc.sync.dma_start(out=xb3_t[:], in_=d_xb3[:])

            def bcast_row(dram, off, w, tag):
                t = res.tile([P, w], f32, tag=tag)
                nc.sync.dma_start(out=t[:], in_=AP(dram, off, [[0, P], [1, w]]))
                return t

            hvbn_b = [bcast_row(d_hvbn, l * P, P, f"hvbn{l}") for l in range(3)]
            bo1_b = bcast_row(d_bo1, 0, P, "bo1")
            bo2_b = bcast_row(d_bo2, 0, D_OUT, "bo2")

            def recip_newton(dst, src):
                r0 = wk.tile(list(dst.shape), f32, tag="rn0")
                nc.vector.reciprocal(r0[:], src)
                t = wk.tile(list(dst.shape), f32, tag="rnt")
                nc.vector.tensor_tensor(out=t[:], in0=src, in1=r0[:],
                                        op=AL.mult)
                nc.vector.tensor_scalar(out=t[:], in0=t[:], scalar1=-1.0,
                                        scalar2=2.0, op0=AL.mult, op1=AL.add)
                nc.vector.tensor_tensor(out=dst, in0=r0[:], in1=t[:],
                                        op=AL.mult)

            x0_res = res.tile([P, NBLK * P], f32)     # x0 + hvbn0
            r_res = res.tile([P, NBLK * P], f32)      # h after layer1 + hvbn2

            def build_u(k2, nch2):
                u_all = up.tile([P, nch2 * P], bf16, tag="U")
                nc.vector.tensor_tensor(
                    out=u_all[:].rearrange("p (c n) -> p c n", n=P),
                    in0=iota_bf[:].unsqueeze(1).to_broadcast([P, nch2, P]),
                    in1=rowrel_t[:, k2:k2 + nch2].unsqueeze(2)
                        .to_broadcast([P, nch2, P]),
                    op=AL.is_equal)
                return u_all

            _nreg_cache = {}

            def nreg(v):
                if v not in _nreg_cache:
                    _nreg_cache[v] = nc.gpsimd.to_reg(v)
                return _nreg_cache[v]

            def gather2(dram, width, kA, nA, kB, nB, tag):
                """Two dma_gathers (lo/hi node windows) into one tile."""
                nch2 = nA + nB
                gt = gp.tile([P, nch2 * width], bf16, tag=tag)
                gv = gt[:].rearrange("p (c w) -> p c w", w=width)
                nc.gpsimd.dma_gather(
                    out_ap=gv[:, 0:nA, :],
                    in_ap=AP(dram, 0, [[width, SPLIT], [1, width]]),
                    idxs_ap=idxa_t[:, kA * 8:(kA + nA) * 8],
                    num_idxs=nA * P, num_idxs_reg=nreg(nA * P),
                    elem_size=width, single_packet=False)
                nc.gpsimd.dma_gather(
                    out_ap=gv[:, nA:nch2, :],
                    in_ap=AP(dram, SPLIT * width,
                             [[width, N - SPLIT], [1, width]]),
                    idxs_ap=idxb_t[:, kB * 8:(kB + nB) * 8],
                    num_idxs=nB * P, num_idxs_reg=nreg(nB * P),
                    elem_size=width, single_packet=False)
                return gt

            # ------------- per-block: build table for layer l -------------
            def build_table(l, b, h_ap):
                v = blk_valid(b)
                nh = NH[l]
                tw2 = 128 + 2 * nh
                tp = pst.tile([P, P], f32, space="PSUM", tag="B")
                nc.tensor.transpose(out=tp[:], in_=h_ap, identity=ident[:])
                hT = wk.tile([P, P], bf16, tag="hT")
                nc.scalar.copy(hT[:], tp[:])
                tabp = ps.tile([P, 144], f32, space="PSUM", tag="A")
                nc.tensor.matmul(out=tabp[:, 0:140], lhsT=hT[:],
                                 rhs=wv_t[l][:], start=True, stop=True)
                tab = wk.tile([P, 140], bf16, tag="tab")
                nc.scalar.copy(tab[:], tabp[:, 0:140])
                nc.vector.memset(tab[:, 128 + nh:128 + 2 * nh], 1.0)
                if l == 1:
                    nc.sync.dma_start(
                        out=ssrc_d[l][b * 128: b * 128 + v, 0:nh],
                        in_=tab[:v, 136:136 + nh])
                else:
                    nc.sync.dma_start(out=srow_d[l][b:b + 1, 0:P],
                                      in_=tab[:P, 136:137])
                nc.sync.dma_start(out=tloc[l][b * 128: b * 128 + v, 0:tw2],
                                  in_=tab[:v, 0:tw2])

            # ------------- output head (after layer 2) -------------
            def out_head(b, h_ap):
                v = blk_valid(b)
                tp = pst.tile([P, P], f32, space="PSUM", tag="B")
                nc.tensor.transpose(out=tp[:], in_=h_ap, identity=ident[:])
                hT = wk.tile([P, P], f32, tag="hTf")
                nc.scalar.copy(hT[:], tp[:])
                t1p = ps.tile([P, 144], f32, space="PSUM", tag="A")
                nc.tensor.matmul(out=t1p[:, 0:P], lhsT=hT[:], rhs=wo1_t[:],
                                 start=True, stop=True)
                t1 = wk.tile([P, P], f32, tag="t1")
                nc.vector.tensor_tensor(out=t1[:], in0=t1p[:, 0:P],
                                        in1=bo1_b[:], op=AL.add)
                nc.scalar.activation(out=t1[:], in_=t1[:], func=AF.Lrelu,
                                     alpha=alpha_t[:])
                tp2 = pst.tile([P, P], f32, space="PSUM", tag="B")
                nc.tensor.transpose(out=tp2[:], in_=t1[:], identity=ident[:])
                t1T = wk.tile([P, P], f32, tag="t1T")
                nc.scalar.copy(t1T[:], tp2[:])
                dp = ps.tile([P, 144], f32, space="PSUM", tag="A")
                nc.tensor.matmul(out=dp[:, 0:D_OUT], lhsT=t1T[:], rhs=wo2_t[:],
                                 start=True, stop=True)
                ot = wk.tile([P, D_OUT], f32, tag="ot")
                nc.vector.tensor_tensor(out=ot[:], in0=dp[:, 0:D_OUT],
                                        in1=bo2_b[:], op=AL.add)
                nc.vector.tensor_tensor(out=ot[:], in0=ot[:],
                                        in1=xb3_t[:, b * 3:(b + 1) * 3],
                                        op=AL.add)
                nc.sync.dma_start(out=d_out[b * 128: b * 128 + v, :],
                                  in_=ot[:v, :])

            # ------------- phase 0: neighbor mean + input proj -------------
            kA = kB = k2 = 0
            for b in range(NBLK):
                nA, nB = SL[b], SH[b]
                nch2 = nA + nB
                gt0 = gather2(d_x128, 128, kA, nA, kB, nB, "gt")
                u_all = build_u(k2, nch2)
                acc = ps.tile([P, 144], f32, space="PSUM", tag="A")
                for s in range(nch2):
                    nc.tensor.matmul(out=acc[:, 0:20],
                                     lhsT=u_all[:, s * P:(s + 1) * P],
                                     rhs=gt0[:, s * 128:s * 128 + 20],
                                     start=(s == 0), stop=(s == nch2 - 1),
                                     skip_group_check=True)
                kA += nA
                kB += nB
                k2 += nch2
                den = wk.tile([P, 1], f32, tag="den")
                nc.vector.tensor_scalar(out=den[:], in0=acc[:, 18:19],
                                        scalar1=1e-8, scalar2=None, op0=AL.add)
                rec = wk.tile([P, 1], f32, tag="rec")
                recip_newton(rec[:], den[:])
                nmean52 = wk.tile([P, 52], f32, tag="nm")
                nc.vector.tensor_scalar(out=nmean52[:, 32:50],
                                        in0=acc[:, 0:D_IN], scalar1=rec[:],
                                        scalar2=None, op0=AL.mult)
                ntp = pst.tile([P, P], f32, space="PSUM", tag="B")
                nc.tensor.transpose(out=ntp[:52, :], in_=nmean52[:],
                                    identity=ident[:])
                lhs = wk.tile([52, P], f32, tag="lhs0")
                nc.vector.memset(lhs[:], 0.0)
                nc.sync.dma_start(out=lhs[0:20, :], in_=d_xT[b, :, :])
                nc.scalar.copy(lhs[32:50, :], ntp[32:50, :])
                h0p = ps.tile([P, 144], f32, space="PSUM", tag="A")
                nc.tensor.matmul(out=h0p[:, 0:P], lhsT=lhs[:], rhs=wcat0_t[:],
                                 start=True, stop=True)
                x0 = wk.tile([P, P], f32, tag="hb")
                nc.scalar.copy(x0[:], h0p[:, 0:P])
                build_table(0, b, x0[:])
                nc.vector.tensor_tensor(out=x0_res[:, b * P:(b + 1) * P],
                                        in0=x0[:], in1=hvbn_b[0][:], op=AL.add)

            def allgather(l, semname):
                import os as _os
                tc.strict_bb_all_engine_barrier()
                if _os.environ.get("KNOCC") != "1":
                    with tc.tile_critical():
                        cc = nc.semaphore(semname).__enter__()
                        nc.gpsimd.collective_compute(
                            "AllGather", AL.bypass,
                            replica_groups=[list(range(NCORES))],
                            ins=[tloc[l].ap().opt()],
                            outs=[tfull[l].ap().opt()],
                        ).then_inc(cc)
                        nc.gpsimd.wait_ge(cc, 1)
                tc.strict_bb_all_engine_barrier()

            allgather(0, "cc0")

            # ------------- attention layers -------------
            def attn_layer(l, resid_res, store_res):
                nh = NH[l]
                tw2 = 128 + 2 * nh
                kA = kB = k2 = 0
                for b in range(NBLK):
                    nA, nB = SL[b], SH[b]
                    nch2 = nA + nB
                    gt = gather2(tfull[l], 256, kA, nA, kB, nB, "gt")
                    gv = gt[:].rearrange("p (c w) -> p c w", w=256)
                    u_all = build_u(k2, nch2)
                    zt = wk.tile([P, nch2 * nh], bf16, tag="zt")
                    if l == 1:
                        sg = gp.tile([P, nch2 * 128], bf16, tag="sg")
                        nc.gpsimd.dma_gather(
                            out_ap=sg[:].rearrange("p (c w) -> p c w", w=128),
                            in_ap=AP(ssrc_d[l], 0, [[128, NSHP], [1, 128]]),
                            idxs_ap=rowloc_t[:, k2 * 8:(k2 + nch2) * 8],
                            num_idxs=nch2 * P, num_idxs_reg=nreg(nch2 * P),
                            elem_size=128, single_packet=False)
                        sv = sg[:].rearrange("p (c w) -> p c w", w=128)
                        nc.vector.tensor_tensor(
                            out=zt[:].rearrange("p (c h) -> p c h", h=4),
                            in0=sv[:, :, 0:4], in1=gv[:, :, 128:132],
                            op=AL.add)
                    else:
                        # srcE via one-hot mult-reduce against the block's
                        # ssrc row (saves a 256B/edge dma_gather)
                        ssrcb = wk.tile([P, P], bf16, tag="ssb")
                        nc.sync.dma_start(
                            out=ssrcb[:],
                            in_=AP(srow_d[l], b * P, [[0, P], [1, P]]))
                        scr = gp.tile([P, nch2 * P], bf16, tag="sg")
                        nc.vector.tensor_tensor(
                            out=scr[:].rearrange("p (c n) -> p c n", n=P),
                            in0=u_all[:].rearrange("p (c n) -> p c n", n=P),
                            in1=ssrcb[:].unsqueeze(1)
                                .to_broadcast([P, nch2, P]),
                            op=AL.mult)
                        se1 = wk.tile([P, nch2], f32, tag="se1")
                        nc.vector.tensor_reduce(
                            out=se1[:],
                            in_=scr[:].rearrange("p (c n) -> p c n", n=P),
                            axis=mybir.AxisListType.X, op=AL.add)
                        se1b = wk.tile([P, nch2], bf16, tag="se1b")
                        nc.vector.tensor_copy(se1b[:], se1[:])
                        nc.vector.tensor_tensor(
                            out=zt[:], in0=se1b[:], in1=gv[:, :, 128],
                            op=AL.add)
                    kA += nA
                    kB += nB
                    k2 += nch2
                    nc.scalar.activation(out=zt[:], in_=zt[:], func=AF.Lrelu,
                                         alpha=alpha_t[:])
                    exb = wk.tile([P, nch2 * nh], bf16, tag="exb")
                    nc.scalar.activation(out=exb[:], in_=zt[:], func=AF.Exp)
                    if l == 1:
                        exv = exb[:].rearrange("p (c h) -> p c h", h=4)
                        exb2 = wk.tile([P, nch2 * 8], bf16, tag="exb2")
                        nc.vector.tensor_copy(
                            exb2[:].rearrange("p (c r h) -> p c r h",
                                              r=2, h=4),
                            exv.unsqueeze(2).to_broadcast([P, nch2, 2, 4]))
                        for h in range(HEADS):
                            nc.vector.tensor_tensor(
                                out=gv[:, :, h * DH:(h + 1) * DH],
                                in0=gv[:, :, h * DH:(h + 1) * DH],
                                in1=exv[:, :, h:h + 1]
                                    .to_broadcast([P, nch2, DH]),
                                op=AL.mult)
                        nc.vector.tensor_tensor(
                            out=gv[:, :, 128:136], in0=gv[:, :, 128:136],
                            in1=exb2[:].rearrange("p (c j) -> p c j", j=8),
                            op=AL.mult)
                    else:
                        nc.vector.tensor_tensor(
                            out=gv[:, :, 0:130], in0=gv[:, :, 0:130],
                            in1=exb[:].unsqueeze(2).to_broadcast(
                                [P, nch2, 130]),
                            op=AL.mult)
                    acc = ps.tile([P, 144], f32, space="PSUM", tag="A")
                    for s in range(nch2):
                        nc.tensor.matmul(out=acc[:, 0:tw2],
                                         lhsT=u_all[:, s * P:(s + 1) * P],
                                         rhs=gt[:, s * 256:s * 256 + tw2],
                                         start=(s == 0), stop=(s == nch2 - 1),
                                         skip_group_check=True)
                    # ----- flush -----
                    den = wk.tile([P, nh], f32, tag="den")
                    nc.vector.tensor_scalar(out=den[:],
                                            in0=acc[:, 128 + nh:128 + 2 * nh],
                                            scalar1=1e-16, scalar2=None,
                                            op0=AL.add)
                    rec = wk.tile([P, nh], f32, tag="rec")
                    recip_newton(rec[:], den[:])
                    hb = wk.tile([P, P], f32, tag="hb")
                    if l == 1:
                        for h in range(HEADS):
                            nc.vector.tensor_scalar(
                                out=hb[:, h * DH:(h + 1) * DH],
                                in0=acc[:, h * DH:(h + 1) * DH],
                                scalar1=rec[:, h:h + 1], scalar2=None,
                                op0=AL.mult)
                    else:
                        nc.vector.tensor_scalar(out=hb[:], in0=acc[:, 0:P],
                                                scalar1=rec[:], scalar2=None,
                                                op0=AL.mult)
                    if resid_res is not None:
                        nc.vector.tensor_tensor(
                            out=hb[:], in0=hb[:],
                            in1=resid_res[:, b * P:(b + 1) * P], op=AL.add)
                    else:
                        nc.vector.tensor_tensor(out=hb[:], in0=hb[:],
                                                in1=hvbn_b[1][:], op=AL.add)
                    nc.scalar.activation(out=hb[:], in_=hb[:], func=AF.Lrelu,
                                         alpha=alpha_t[:])
                    if l < 2:
                        build_table(l + 1, b, hb[:])
                    else:
                        out_head(b, hb[:])
                    if store_res is not None:
                        nc.vector.tensor_tensor(
                            out=store_res[:, b * P:(b + 1) * P], in0=hb[:],
                            in1=hvbn_b[2][:], op=AL.add)

            import os
            _bis = os.environ.get("KBISECT", "full")
            if _bis == "p0":
                zz = res.tile([P, D_OUT], f32)
                nc.vector.memset(zz[:], 0.0)
                for b in range(NBLK):
                    v = blk_valid(b)
                    nc.sync.dma_start(out=d_out[b * 128: b * 128 + v, :],
                                      in_=zz[:v, :])
            else:
                attn_layer(0, x0_res, None)
                allgather(1, "cc1")
                if _bis == "l0":
                    zz = res.tile([P, D_OUT], f32)
                    nc.vector.memset(zz[:], 0.0)
                    for b in range(NBLK):
                        v = blk_valid(b)
                        nc.sync.dma_start(out=d_out[b * 128: b * 128 + v, :],
                                          in_=zz[:v, :])
                else:
                    attn_layer(1, None, r_res)
                    allgather(2, "cc2")
                    if _bis == "l1":
                        zz = res.tile([P, D_OUT], f32)
                        nc.vector.memset(zz[:], 0.0)
                        for b in range(NBLK):
                            v = blk_valid(b)
                            nc.sync.dma_start(
                                out=d_out[b * 128: b * 128 + v, :],
                                in_=zz[:v, :])
                    else:
                        attn_layer(2, r_res, None)

    _split_multi_waits(nc, 1)
    lower_extended_insts(nc)
    return nc


class tile_pools:
    """All pools opened/closed together."""

    def __init__(self, tc):
        self.tc = tc

    def __enter__(self):
        tc = self.tc
        self.cms = [
            tc.tile_pool(name="res", bufs=1),
            tc.tile_pool(name="wk", bufs=3),
            tc.tile_pool(name="gat", bufs=3),
            tc.tile_pool(name="u", bufs=3),
            tc.tile_pool(name="ps", bufs=5, space="PSUM"),
            tc.tile_pool(name="pst", bufs=3, space="PSUM"),
        ]
        return tuple(cm.__enter__() for cm in self.cms)

    def __exit__(self, *a):
        for cm in reversed(self.cms):
            cm.__exit__(*a)
        return False


# ---------------------------------------------------------------------------
# execution: replicate bass2jax.run_bass_via_pjrt's multi-core path, but keep
# a handle on the compiled executable + inputs so a bench harness can re-time
# repeated executions without recompiling.
# ---------------------------------------------------------------------------

_LAST_RUN = None


def _run_spmd_keep(nc, in_maps, n_cores):
    global _LAST_RUN
    import jax
    from jax.experimental.shard_map import shard_map
    from jax.sharding import Mesh, PartitionSpec

    from concourse import bass2jax

    bass2jax.install_neuronx_cc_hook()
    assert nc.dbg_addr is None
    partition_name = (nc.partition_id_tensor.name
                      if nc.partition_id_tensor else None)

    in_names, out_names, out_avals, zero_outs = [], [], [], []
    for alloc in nc.m.functions[0].allocations:
        if not isinstance(alloc, mybir.MemoryLocationSet):
            continue
        name = alloc.memorylocations[0].name
        if alloc.kind == "ExternalInput":
            if name != partition_name:
                in_names.append(name)
        elif alloc.kind == "ExternalOutput":
            shape = tuple(alloc.tensor_shape)
            dtype = mybir.dt.np(alloc.dtype)
            out_names.append(name)
            out_avals.append(jax.core.ShapedArray(shape, dtype))
            zero_outs.append(np.zeros(shape, dtype))
    n_params = len(in_names)
    n_outs = len(out_avals)
    all_names = in_names + out_names
    if partition_name is not None:
        all_names = all_names + [partition_name]
    donate = tuple(range(n_params, n_params + n_outs))

    def _body(*args):
        operands = list(args)
        if partition_name is not None:
            operands.append(bass2jax.partition_id_tensor())
        outs = bass2jax._bass_exec_p.bind(
            *operands,
            out_avals=tuple(out_avals),
            in_names=tuple(all_names),
            out_names=tuple(out_names),
            lowering_input_output_aliases=(),
            sim_require_finite=True,
            sim_require_nnan=True,
            nc=nc,
        )
        return tuple(outs)

    devices = jax.devices()[:n_cores]
    assert len(devices) == n_cores
    mesh = Mesh(np.asarray(devices), ("core",))
    in_specs = (PartitionSpec("core"),) * (n_params + n_outs)
    out_specs = (PartitionSpec("core"),) * n_outs
    sharded = jax.jit(
        shard_map(_body, mesh=mesh, in_specs=in_specs, out_specs=out_specs,
                  check_rep=False),
        donate_argnums=donate, keep_unused=True,
    )
    per_core = [[np.asarray(m[name]) for name in in_names] for m in in_maps]
    concat_in = [
        np.concatenate([per_core[c][i] for c in range(n_cores)], axis=0)
        for i in range(n_params)
    ]
    concat_zeros = [
        np.zeros((n_cores * z.shape[0], *z.shape[1:]), z.dtype)
        for z in zero_outs
    ]
    out_arrs = sharded(*concat_in, *concat_zeros)
    _LAST_RUN = dict(
        sharded=sharded, concat_in=concat_in, mesh=mesh,
        zero_specs=[(z.shape, z.dtype) for z in concat_zeros],
        out_names=out_names, out_avals=out_avals, n_cores=n_cores,
    )
    return [
        {
            name: np.asarray(out_arrs[i]).reshape(n_cores, *out_avals[i].shape)[c]
            for i, name in enumerate(out_names)
        }
        for c in range(n_cores)
    ]


# ---------------------------------------------------------------------------
# public entry point
# ---------------------------------------------------------------------------

def kernel(**inputs):
    _apply_patches()
    import ml_dtypes
    bf = ml_dtypes.bfloat16

    x = np.asarray(inputs["x"], np.float32)
    edge_index = np.asarray(inputs["edge_index"], np.int32)

    SL, SH, SLt, SHt, C2, rowrel, idxa, idxb, rowloc = _preprocess(edge_index)

    W_in = np.asarray(inputs["W_in"], np.float32)
    b_in = np.asarray(inputs["b_in"], np.float32)
    W_agg = np.asarray(inputs["W_agg"], np.float32)
    b_agg = np.asarray(inputs["b_agg"], np.float32)
    sh_Wv = np.asarray(inputs["sh_Wv"], np.float32)
    sh_b = np.asarray(inputs["sh_b"], np.float32)
    sh_asrc = np.asarray(inputs["sh_asrc"], np.float32)
    sh_adst = np.asarray(inputs["sh_adst"], np.float32)
    mh_Wv = np.asarray(inputs["mh_Wv"], np.float32)
    mh_b = np.asarray(inputs["mh_b"], np.float32)
    mh_asrc = np.asarray(inputs["mh_asrc"], np.float32)
    mh_adst = np.asarray(inputs["mh_adst"], np.float32)
    bn_g = np.asarray(inputs["bn_gamma"], np.float32)
    bn_b = np.asarray(inputs["bn_beta"], np.float32)
    bn_m = np.asarray(inputs["bn_mean"], np.float32)
    bn_v = np.asarray(inputs["bn_var"], np.float32)
    W_o1 = np.asarray(inputs["W_o1"], np.float32)
    b_o1 = np.asarray(inputs["b_o1"], np.float32)
    W_o2 = np.asarray(inputs["W_o2"], np.float32)
    b_o2 = np.asarray(inputs["b_o2"], np.float32)

    bnsc = (bn_g / np.sqrt(bn_v + BN_EPS)).astype(np.float32)
    bnsh = (bn_b - bn_m * bnsc).astype(np.float32)

    # wcat0 rows pair with lhsT rows: 0:18 x | 18 ones | 32:50 nmean
    wcat0 = np.zeros((52, P), np.float32)
    wcat0[0:18] = W_in
    wcat0[18] = b_in + b_agg
    wcat0[32:50] = W_agg

    # table-build weights: cols 0:128 BN-folded values | 128:128+nh a_dst |
    # 128+nh:128+2nh zeros (ones via memset) | 136:136+nh a_src
    wv = np.zeros((3, P, 140), np.float32)
    wv[0, :, 0:P] = sh_Wv[0] * bnsc[0][None, :]
    wv[0, :, 128] = sh_adst[0]
    wv[0, :, 136] = sh_asrc[0]
    wv[1, :, 0:P] = mh_Wv.transpose(1, 0, 2).reshape(P, P) * bnsc[1][None, :]
    wv[1, :, 128:132] = mh_adst.T
    wv[1, :, 136:140] = mh_asrc.T
    wv[2, :, 0:P] = sh_Wv[1] * bnsc[2][None, :]
    wv[2, :, 128] = sh_adst[1]
    wv[2, :, 136] = sh_asrc[1]

    # value bias (BN-folded) + BN shift, applied post-softmax via residuals
    hvbn = np.zeros((3, P), np.float32)
    hvbn[0] = sh_b[0] * bnsc[0] + bnsh[0]
    hvbn[1] = mh_b.reshape(P) * bnsc[1] + bnsh[1]
    hvbn[2] = sh_b[1] * bnsc[2] + bnsh[2]

    x128 = np.zeros((N, 128), np.float32)
    x128[:, :D_IN] = x
    x128[:, D_IN] = 1.0

    xT = np.zeros((NCORES, NBLK, 20, P), np.float32)
    xb3 = np.zeros((NCORES, P, NBLK * 3), np.float32)
    for ci in range(NCORES):
        shp = np.zeros((NBLK * 128, 20), np.float32)
        shp[:NSH] = x128[ci * NSH:(ci + 1) * NSH, 0:20]
        xT[ci] = shp.reshape(NBLK, 128, 20).transpose(0, 2, 1)
        x3 = np.zeros((NBLK * 128, 3), np.float32)
        x3[:NSH] = x[ci * NSH:(ci + 1) * NSH, -3:]
        xb3[ci] = x3.reshape(NBLK, 128, 3).transpose(1, 0, 2).reshape(
            P, NBLK * 3)

    nc = _build(SL, SH, SLt, SHt, C2)

    in_maps = []
    for ci in range(NCORES):
        in_maps.append({
            "rowrel": rowrel[ci], "idxa": idxa[ci], "idxb": idxb[ci],
            "rowloc": rowloc[ci],
            "x128": x128.astype(bf), "xT": xT[ci], "xb3": xb3[ci],
            "wcat0": wcat0, "wv": wv.astype(bf), "hvbn": hvbn,
            "wo1": W_o1, "bo1": b_o1[None, :], "wo2": W_o2,
            "bo2": b_o2[None, :],
        })

    res = _run_spmd_keep(nc, in_maps, NCORES)
    out = np.concatenate([res[ci]["out"] for ci in range(NCORES)], axis=0)
    return out.astype(np.float32)


# revision 16
# speedup vs baseline: 2.0439x; 2.0439x over previous
"""EnhancedFlowGNN forward pass on 8 Trainium2 NeuronCores (Bass/Tile), v2.

Strategy (edge parallelism aligned with a node partition, no all-reduce):
  - Host sorts edges by destination ("row") and shards them by row range so
    core i owns nodes [i*6250, (i+1)*6250) and every edge targeting them.
  - All per-edge data movement is BULK: per 128-dest-node block, gpsimd
    dma_gather instructions fetch every edge's table row (node table split
    into two <32768-row windows so indices fit int16) and every edge's
    ssrc[row] — 3 gather instructions per block instead of one indirect DMA
    per 128-edge chunk.
  - All per-edge compute is batched per block: one DVE op builds every
    one-hot scatter matrix, one (or five) DVE ops apply the softmax weights
    in-place on the gathered rows. Only the PSUM-accumulating scatter
    matmuls remain per-chunk.
  - Softmax denominator arrives through ones-columns in the table; no
    segment_max (|logits| stay O(1) for this model family) and no per-edge
    alpha materialization.
  - BN scale is folded into the value weights host-side; BN shift + value
    bias are folded into the residual streams post-softmax.
  - The gather/softmax/scatter pipeline runs in bf16 (output = x[:,-3:] +
    small delta, so datapath errors land ~1e-4 relative).
"""

import numpy as np

import concourse.bass as bass
import concourse.mybir as mybir
import concourse.tile as tile
from concourse.bass import AP
from concourse.tile import ScopedClock

f32 = mybir.dt.float32
bf16 = mybir.dt.bfloat16
i16 = mybir.dt.int16
i32 = mybir.dt.int32

N = 50000
E = 800000
D_IN = 18
H = 128
HEADS = 4
DH = H // HEADS
D_OUT = 3
NEG = 0.2
BN_EPS = 1e-5

NCORES = 8
NSH = N // NCORES            # 6250 nodes per core
NBLK = (NSH + 127) // 128    # 49 blocks (48 full + one of 106)
NSHP = NBLK * 128            # padded node count (ssrc table rows)
P = 128
SPLIT = 32768                # node-table window split (int16 index limit)

NH = (1, 4, 1)               # heads per attention layer
# gathered row: [vals 0:128 | sdst 128:128+nh | ones .. 128+2nh | .. 256]
# table-build matmul also emits ssrc at cols 136:136+nh


# ---------------------------------------------------------------------------
# container compat patches (older walrus in this image)
# ---------------------------------------------------------------------------

_patched = False


def _apply_patches():
    global _patched
    if _patched:
        return
    _patched = True

    from concourse.bass import compact_to_ranges

    # The walrus here accepts at most ONE sync-wait command per instruction,
    # and the EVSEM range-clear in the Tile tail lowers to an InstISA
    # encoding it rejects. Each kernel() call builds + loads a fresh NEFF,
    # so semaphores start zeroed and the tail clears can be dropped.
    def _drain_and_barrier(self, tick_clock, wait_clock):
        nc = self.nc
        drain_inst = nc.sync.drain()
        wait_clock.add_sem_waits(
            drain_inst.ins, ScopedClock({None: tick_clock.global_clock})
        )
        nc.all_engine_barrier()
        popped = nc._tile_sem_poison_stack.pop()
        assert popped is self._sem_poison
        sems = list(self.sems.allocated().values())
        if sems:
            sem_nums = [
                s.num if isinstance(s, bass.SemaphoreHandle) else s for s in sems
            ]
            for sem_range in compact_to_ranges(sem_nums):
                nc.gpsimd.dma_reset(sem_range)
            nc._state.prepend_free_semaphores(sem_nums)
            for poison_set in nc._tile_sem_poison_stack:
                poison_set.update(sem_nums)
        nc.all_engine_barrier()

    tile.TileContext._drain_and_barrier = _drain_and_barrier


_WAITSPLIT_CTR = [0]


def _split_multi_waits(nc, max_waits=1):
    """Move extra sync waits onto same-engine NoOps (walrus limit: 1/inst)."""
    for f in nc.m.functions:
        for b in f.blocks:
            insts = b.instructions
            i = 0
            while i < len(insts):
                inst = insts[i]
                si = inst.sync_info
                if si is not None:
                    waits = list(si.on_wait)
                    imm = [w for w in waits if w.wait_reg is None]
                    reg = [w for w in waits if w.wait_reg is not None]
                    budget = max(0, max_waits - len(reg))
                    if len(imm) > budget:
                        keep = imm[len(imm) - budget:] if budget else []
                        extras = imm[: len(imm) - budget]
                        si.on_wait = reg + keep
                        for j in range(0, len(extras), max_waits):
                            _WAITSPLIT_CTR[0] += 1
                            nop = mybir.InstNoOp(
                                name=f"I-waitsplit-{_WAITSPLIT_CTR[0]}"
                            )
                            nop.engine = inst.engine
                            nop.sync_info = mybir.SyncInfo(
                                on_wait=extras[j: j + max_waits], on_update=[]
                            )
                            insts.insert(i, nop)
                            i += 1
                i += 1


# ---------------------------------------------------------------------------
# host-side preprocessing
# ---------------------------------------------------------------------------

def _pack16(arr):
    """Wrap a flat int list into the dma_gather index layout: index i at
    partition i%16, slot i//16, replicated to all 8 Q7-core groups."""
    w = arr.reshape(-1, 16).T.astype(np.int16)      # [16, n/16]
    return np.tile(w, (8, 1))                       # [128, n/16]


def _preprocess(edge_index):
    row = edge_index[0].astype(np.int64)
    col = edge_index[1].astype(np.int64)
    order = np.argsort(row, kind="stable")
    rs, cs = row[order], col[order]

    per_core = []
    SL = np.zeros(NBLK, np.int64)
    SH = np.zeros(NBLK, np.int64)
    for ci in range(NCORES):
        lo = np.searchsorted(rs, ci * NSH, "left")
        hi = np.searchsorted(rs, (ci + 1) * NSH, "left")
        r = rs[lo:hi] - ci * NSH
        c = cs[lo:hi]
        blocks = []
        for b in range(NBLK):
            blo = np.searchsorted(r, b * 128, "left")
            bhi = np.searchsorted(r, min((b + 1) * 128, NSH), "left")
            rr = r[blo:bhi] - b * 128
            cc = c[blo:bhi]
            o2 = np.argsort(cc, kind="stable")   # ascending-address gathers
            rr, cc = rr[o2], cc[o2]
            m = cc < SPLIT
            blocks.append(((rr[m], cc[m]), (rr[~m], cc[~m] - SPLIT)))
            SL[b] = max(SL[b], (int(m.sum()) + 127) // 128)
            SH[b] = max(SH[b], (int((~m).sum()) + 127) // 128)
        per_core.append(blocks)

    SL = np.maximum(SL, 1)
    SH = np.maximum(SH, 1)
    SLt, SHt = int(SL.sum()), int(SH.sum())
    C2 = SLt + SHt

    import ml_dtypes
    rowrel = np.full((NCORES, P, C2), -1.0, np.float32)
    idxa = np.zeros((NCORES, P, SLt * 8), np.int16)
    idxb = np.zeros((NCORES, P, SHt * 8), np.int16)
    rowloc = np.zeros((NCORES, P, C2 * 8), np.int16)
    for ci in range(NCORES):
        kA = kB = k2 = 0
        for b in range(NBLK):
            (rl, cl), (rh, ch) = per_core[ci][b]
            nL, nH = SL[b] * 128, SH[b] * 128
            fl = np.zeros(nL, np.int64)
            fl[:len(cl)] = cl
            fh = np.zeros(nH, np.int64)
            fh[:len(ch)] = ch
            rloc = np.zeros(nL + nH, np.int64)
            rloc[:len(rl)] = b * 128 + rl
            rloc[nL:nL + len(rh)] = b * 128 + rh
            idxa[ci, :, kA * 8:(kA + SL[b]) * 8] = _pack16(fl)
            idxb[ci, :, kB * 8:(kB + SH[b]) * 8] = _pack16(fh)
            rowloc[ci, :, k2 * 8:(k2 + SL[b] + SH[b]) * 8] = _pack16(rloc)
            rr2 = np.full(nL + nH, -1.0, np.float32)
            rr2[:len(rl)] = rl
            rr2[nL:nL + len(rh)] = rh
            rowrel[ci, :, k2:k2 + SL[b] + SH[b]] = rr2.reshape(-1, 128).T
            kA += SL[b]
            kB += SH[b]
            k2 += SL[b] + SH[b]
        assert kA == SLt and kB == SHt and k2 == C2

    return (list(SL), list(SH), SLt, SHt, C2,
            rowrel.astype(ml_dtypes.bfloat16), idxa, idxb, rowloc)


# ---------------------------------------------------------------------------
# device kernel
# ---------------------------------------------------------------------------

def _build(SL, SH, SLt, SHt, C2):
    from concourse import library_config
    from concourse.library_overlay import lower_extended_insts

    nc = bass.Bass("TRN2", target_bir_lowering=False)

    d_rowrel = nc.dram_tensor("rowrel", [P, C2], bf16, kind="ExternalInput")
    d_idxa = nc.dram_tensor("idxa", [P, SLt * 8], i16, kind="ExternalInput")
    d_idxb = nc.dram_tensor("idxb", [P, SHt * 8], i16, kind="ExternalInput")
    d_rowloc = nc.dram_tensor("rowloc", [P, C2 * 8], i16,
                              kind="ExternalInput")
    d_x128 = nc.dram_tensor("x128", [N, 128], bf16, kind="ExternalInput")
    d_xT = nc.dram_tensor("xT", [NBLK, 20, P], f32, kind="ExternalInput")
    d_xb3 = nc.dram_tensor("xb3", [P, NBLK * 3], f32, kind="ExternalInput")
    d_wcat0 = nc.dram_tensor("wcat0", [52, P], f32, kind="ExternalInput")
    d_wv = nc.dram_tensor("wv", [3, P, 140], bf16, kind="ExternalInput")
    d_hvbn = nc.dram_tensor("hvbn", [3, P], f32, kind="ExternalInput")
    d_wo1 = nc.dram_tensor("wo1", [P, P], f32, kind="ExternalInput")
    d_bo1 = nc.dram_tensor("bo1", [1, P], f32, kind="ExternalInput")
    d_wo2 = nc.dram_tensor("wo2", [P, D_OUT], f32, kind="ExternalInput")
    d_bo2 = nc.dram_tensor("bo2", [1, D_OUT], f32, kind="ExternalInput")
    d_out = nc.dram_tensor("out", [NSH, D_OUT], f32, kind="ExternalOutput")

    tloc = [nc.dram_tensor(f"tloc{l}", [NSH, 256], bf16) for l in range(3)]
    tfull = [nc.dram_tensor(f"tfull{l}", [N, 256], bf16,
                            addr_space="Shared") for l in range(3)]
    ssrc_d = [nc.dram_tensor(f"ssrc{l}", [NSHP, 128], bf16) for l in range(3)]

    AL = mybir.AluOpType
    AF = mybir.ActivationFunctionType

    def blk_valid(b):
        return P if b < NBLK - 1 else NSH - (NBLK - 1) * 128

    with tile.TileContext(nc) as tc:
        with tile_pools(tc) as (res, wk, gp, up, ps, pst):

            # ---- constants / resident tiles (gpsimd iota BEFORE the
            # library reload; no gpsimd compute afterwards) ----
            # explicit standard-library load first: a re-execution of this
            # NEFF starts with the mlp library still resident from the
            # previous run, and iota needs standard ucode
            nc.gpsimd.load_library(library_config.standard)
            iota_i = res.tile([P, P], i32)
            nc.gpsimd.iota(iota_i[:], pattern=[[1, P]], base=0,
                           channel_multiplier=0)
            iop_i = res.tile([P, 1], i32)
            nc.gpsimd.iota(iop_i[:], pattern=[[0, 1]], base=0,
                           channel_multiplier=1)
            nc.gpsimd.load_library(library_config.mlp)

            iota_f = res.tile([P, P], f32)
            nc.vector.tensor_copy(iota_f[:], iota_i[:])
            iota_bf = res.tile([P, P], bf16)
            nc.vector.tensor_copy(iota_bf[:], iota_i[:])
            iop_f = res.tile([P, 1], f32)
            nc.vector.tensor_copy(iop_f[:], iop_i[:])
            ident = res.tile([P, P], f32)
            nc.vector.tensor_scalar(out=ident[:], in0=iota_f[:],
                                    scalar1=iop_f[:], scalar2=None,
                                    op0=AL.is_equal)
            alpha_t = res.tile([P, 1], f32)
            nc.vector.memset(alpha_t[:], NEG)

            rowrel_t = res.tile([P, C2], bf16)
            nc.sync.dma_start(out=rowrel_t[:], in_=d_rowrel[:])
            idxa_t = res.tile([P, SLt * 8], i16)
            nc.sync.dma_start(out=idxa_t[:], in_=d_idxa[:])
            idxb_t = res.tile([P, SHt * 8], i16)
            nc.sync.dma_start(out=idxb_t[:], in_=d_idxb[:])
            rowloc_t = res.tile([P, C2 * 8], i16)
            nc.sync.dma_start(out=rowloc_t[:], in_=d_rowloc[:])
            wcat0_t = res.tile([52, P], f32)
            nc.sync.dma_start(out=wcat0_t[:], in_=d_wcat0[:])
            wv_t = []
            for l in range(3):
                wvl = res.tile([P, 140], bf16, tag=f"wv{l}")
                wv_t.append(wvl)
            for l in range(3):
                nc.sync.dma_start(out=wv_t[l][:], in_=d_wv[l, :, :])
            wo1_t = res.tile([P, P], f32)
            nc.sync.dma_start(out=wo1_t[:], in_=d_wo1[:])
            wo2_t = res.tile([P, D_OUT], f32)
            nc.sync.dma_start(out=wo2_t[:], in_=d_wo2[:])
            xb3_t = res.tile([P, NBLK * 3], f32)
            nc.sync.dma_start(out=xb3_t[:], in_=d_xb3[:])

            def bcast_row(dram, off, w, tag):
                t = res.tile([P, w], f32, tag=tag)
                nc.sync.dma_start(out=t[:], in_=AP(dram, off, [[0, P], [1, w]]))
                return t

            hvbn_b = [bcast_row(d_hvbn, l * P, P, f"hvbn{l}") for l in range(3)]
            bo1_b = bcast_row(d_bo1, 0, P, "bo1")
            bo2_b = bcast_row(d_bo2, 0, D_OUT, "bo2")

            def recip_newton(dst, src):
                r0 = wk.tile(list(dst.shape), f32, tag="rn0")
                nc.vector.reciprocal(r0[:], src)
                t = wk.tile(list(dst.shape), f32, tag="rnt")
                nc.vector.tensor_tensor(out=t[:], in0=src, in1=r0[:],
                                        op=AL.mult)
                nc.vector.tensor_scalar(out=t[:], in0=t[:], scalar1=-1.0,
                                        scalar2=2.0, op0=AL.mult, op1=AL.add)
                nc.vector.tensor_tensor(out=dst, in0=r0[:], in1=t[:],
                                        op=AL.mult)

            x0_res = res.tile([P, NBLK * P], f32)     # x0 + hvbn0
            r_res = res.tile([P, NBLK * P], f32)      # h after layer1 + hvbn2

            def build_u(k2, nch2):
                u_all = up.tile([P, nch2 * P], bf16, tag="U")
                nc.vector.tensor_tensor(
                    out=u_all[:].rearrange("p (c n) -> p c n", n=P),
                    in0=iota_bf[:].unsqueeze(1).to_broadcast([P, nch2, P]),
                    in1=rowrel_t[:, k2:k2 + nch2].unsqueeze(2)
                        .to_broadcast([P, nch2, P]),
                    op=AL.is_equal)
                return u_all

            _nreg_cache = {}

            def nreg(v):
                if v not in _nreg_cache:
                    _nreg_cache[v] = nc.gpsimd.to_reg(v)
                return _nreg_cache[v]

            def gather2(dram, width, kA, nA, kB, nB, tag):
                """Two dma_gathers (lo/hi node windows) into one tile."""
                nch2 = nA + nB
                gt = gp.tile([P, nch2 * width], bf16, tag=tag)
                gv = gt[:].rearrange("p (c w) -> p c w", w=width)
                nc.gpsimd.dma_gather(
                    out_ap=gv[:, 0:nA, :],
                    in_ap=AP(dram, 0, [[width, SPLIT], [1, width]]),
                    idxs_ap=idxa_t[:, kA * 8:(kA + nA) * 8],
                    num_idxs=nA * P, num_idxs_reg=nreg(nA * P),
                    elem_size=width, single_packet=False)
                nc.gpsimd.dma_gather(
                    out_ap=gv[:, nA:nch2, :],
                    in_ap=AP(dram, SPLIT * width,
                             [[width, N - SPLIT], [1, width]]),
                    idxs_ap=idxb_t[:, kB * 8:(kB + nB) * 8],
                    num_idxs=nB * P, num_idxs_reg=nreg(nB * P),
                    elem_size=width, single_packet=False)
                return gt

            # ------------- per-block: build table for layer l -------------
            def build_table(l, b, h_ap):
                v = blk_valid(b)
                nh = NH[l]
                tw2 = 128 + 2 * nh
                tp = pst.tile([P, P], f32, space="PSUM", tag="B")
                nc.tensor.transpose(out=tp[:], in_=h_ap, identity=ident[:])
                hT = wk.tile([P, P], bf16, tag="hT")
                nc.scalar.copy(hT[:], tp[:])
                tabp = ps.tile([P, 144], f32, space="PSUM", tag="A")
                nc.tensor.matmul(out=tabp[:, 0:140], lhsT=hT[:],
                                 rhs=wv_t[l][:], start=True, stop=True)
                tab = wk.tile([P, 140], bf16, tag="tab")
                nc.scalar.copy(tab[:], tabp[:, 0:140])
                nc.vector.memset(tab[:, 128 + nh:128 + 2 * nh], 1.0)
                nc.sync.dma_start(out=ssrc_d[l][b * 128: b * 128 + v, 0:nh],
                                  in_=tab[:v, 136:136 + nh])
                nc.sync.dma_start(out=tloc[l][b * 128: b * 128 + v, 0:tw2],
                                  in_=tab[:v, 0:tw2])

            # ------------- output head (after layer 2) -------------
            def out_head(b, h_ap):
                v = blk_valid(b)
                tp = pst.tile([P, P], f32, space="PSUM", tag="B")
                nc.tensor.transpose(out=tp[:], in_=h_ap, identity=ident[:])
                hT = wk.tile([P, P], f32, tag="hTf")
                nc.scalar.copy(hT[:], tp[:])
                t1p = ps.tile([P, 144], f32, space="PSUM", tag="A")
                nc.tensor.matmul(out=t1p[:, 0:P], lhsT=hT[:], rhs=wo1_t[:],
                                 start=True, stop=True)
                t1 = wk.tile([P, P], f32, tag="t1")
                nc.vector.tensor_tensor(out=t1[:], in0=t1p[:, 0:P],
                                        in1=bo1_b[:], op=AL.add)
                nc.scalar.activation(out=t1[:], in_=t1[:], func=AF.Lrelu,
                                     alpha=alpha_t[:])
                tp2 = pst.tile([P, P], f32, space="PSUM", tag="B")
                nc.tensor.transpose(out=tp2[:], in_=t1[:], identity=ident[:])
                t1T = wk.tile([P, P], f32, tag="t1T")
                nc.scalar.copy(t1T[:], tp2[:])
                dp = ps.tile([P, 144], f32, space="PSUM", tag="A")
                nc.tensor.matmul(out=dp[:, 0:D_OUT], lhsT=t1T[:], rhs=wo2_t[:],
                                 start=True, stop=True)
                ot = wk.tile([P, D_OUT], f32, tag="ot")
                nc.vector.tensor_tensor(out=ot[:], in0=dp[:, 0:D_OUT],
                                        in1=bo2_b[:], op=AL.add)
                nc.vector.tensor_tensor(out=ot[:], in0=ot[:],
                                        in1=xb3_t[:, b * 3:(b + 1) * 3],
                                        op=AL.add)
                nc.sync.dma_start(out=d_out[b * 128: b * 128 + v, :],
                                  in_=ot[:v, :])

            # ------------- phase 0: neighbor mean + input proj -------------
            kA = kB = k2 = 0
            for b in range(NBLK):
                nA, nB = SL[b], SH[b]
                nch2 = nA + nB
                gt0 = gather2(d_x128, 128, kA, nA, kB, nB, "gt")
                u_all = build_u(k2, nch2)
                acc = ps.tile([P, 144], f32, space="PSUM", tag="A")
                for s in range(nch2):
                    nc.tensor.matmul(out=acc[:, 0:20],
                                     lhsT=u_all[:, s * P:(s + 1) * P],
                                     rhs=gt0[:, s * 128:s * 128 + 20],
                                     start=(s == 0), stop=(s == nch2 - 1),
                                     skip_group_check=True)
                kA += nA
                kB += nB
                k2 += nch2
                den = wk.tile([P, 1], f32, tag="den")
                nc.vector.tensor_scalar(out=den[:], in0=acc[:, 18:19],
                                        scalar1=1e-8, scalar2=None, op0=AL.add)
                rec = wk.tile([P, 1], f32, tag="rec")
                recip_newton(rec[:], den[:])
                nmean52 = wk.tile([P, 52], f32, tag="nm")
                nc.vector.tensor_scalar(out=nmean52[:, 32:50],
                                        in0=acc[:, 0:D_IN], scalar1=rec[:],
                                        scalar2=None, op0=AL.mult)
                ntp = pst.tile([P, P], f32, space="PSUM", tag="B")
                nc.tensor.transpose(out=ntp[:52, :], in_=nmean52[:],
                                    identity=ident[:])
                lhs = wk.tile([52, P], f32, tag="lhs0")
                nc.vector.memset(lhs[:], 0.0)
                nc.sync.dma_start(out=lhs[0:20, :], in_=d_xT[b, :, :])
                nc.scalar.copy(lhs[32:50, :], ntp[32:50, :])
                h0p = ps.tile([P, 144], f32, space="PSUM", tag="A")
                nc.tensor.matmul(out=h0p[:, 0:P], lhsT=lhs[:], rhs=wcat0_t[:],
                                 start=True, stop=True)
                x0 = wk.tile([P, P], f32, tag="hb")
                nc.scalar.copy(x0[:], h0p[:, 0:P])
                build_table(0, b, x0[:])
                nc.vector.tensor_tensor(out=x0_res[:, b * P:(b + 1) * P],
                                        in0=x0[:], in1=hvbn_b[0][:], op=AL.add)

            def allgather(l, semname):
                import os as _os
                tc.strict_bb_all_engine_barrier()
                if _os.environ.get("KNOCC") != "1":
                    with tc.tile_critical():
                        cc = nc.semaphore(semname).__enter__()
                        nc.gpsimd.collective_compute(
                            "AllGather", AL.bypass,
                            replica_groups=[list(range(NCORES))],
                            ins=[tloc[l].ap().opt()],
                            outs=[tfull[l].ap().opt()],
                        ).then_inc(cc)
                        nc.gpsimd.wait_ge(cc, 1)
                tc.strict_bb_all_engine_barrier()

            allgather(0, "cc0")

            # ------------- attention layers -------------
            def attn_layer(l, resid_res, store_res):
                nh = NH[l]
                tw2 = 128 + 2 * nh
                kA = kB = k2 = 0
                for b in range(NBLK):
                    nA, nB = SL[b], SH[b]
                    nch2 = nA + nB
                    gt = gather2(tfull[l], 256, kA, nA, kB, nB, "gt")
                    gv = gt[:].rearrange("p (c w) -> p c w", w=256)
                    sg = gp.tile([P, nch2 * 128], bf16, tag="sg")
                    nc.gpsimd.dma_gather(
                        out_ap=sg[:].rearrange("p (c w) -> p c w", w=128),
                        in_ap=AP(ssrc_d[l], 0, [[128, NSHP], [1, 128]]),
                        idxs_ap=rowloc_t[:, k2 * 8:(k2 + nch2) * 8],
                        num_idxs=nch2 * P, num_idxs_reg=nreg(nch2 * P),
                        elem_size=128, single_packet=False)
                    sv = sg[:].rearrange("p (c w) -> p c w", w=128)
                    u_all = build_u(k2, nch2)
                    kA += nA
                    kB += nB
                    k2 += nch2
                    zt = wk.tile([P, nch2 * nh], bf16, tag="zt")
                    if l == 1:
                        nc.vector.tensor_tensor(
                            out=zt[:].rearrange("p (c h) -> p c h", h=4),
                            in0=sv[:, :, 0:4], in1=gv[:, :, 128:132],
                            op=AL.add)
                    else:
                        nc.vector.tensor_tensor(
                            out=zt[:], in0=sv[:, :, 0], in1=gv[:, :, 128],
                            op=AL.add)
                    nc.scalar.activation(out=zt[:], in_=zt[:], func=AF.Lrelu,
                                         alpha=alpha_t[:])
                    exb = wk.tile([P, nch2 * nh], bf16, tag="exb")
                    nc.scalar.activation(out=exb[:], in_=zt[:], func=AF.Exp)
                    if l == 1:
                        exv = exb[:].rearrange("p (c h) -> p c h", h=4)
                        exb2 = wk.tile([P, nch2 * 8], bf16, tag="exb2")
                        nc.vector.tensor_copy(
                            exb2[:].rearrange("p (c r h) -> p c r h",
                                              r=2, h=4),
                            exv.unsqueeze(2).to_broadcast([P, nch2, 2, 4]))
                        for h in range(HEADS):
                            nc.vector.tensor_tensor(
                                out=gv[:, :, h * DH:(h + 1) * DH],
                                in0=gv[:, :, h * DH:(h + 1) * DH],
                                in1=exv[:, :, h:h + 1]
                                    .to_broadcast([P, nch2, DH]),
                                op=AL.mult)
                        nc.vector.tensor_tensor(
                            out=gv[:, :, 128:136], in0=gv[:, :, 128:136],
                            in1=exb2[:].rearrange("p (c j) -> p c j", j=8),
                            op=AL.mult)
                    else:
                        nc.vector.tensor_tensor(
                            out=gv[:, :, 0:130], in0=gv[:, :, 0:130],
                            in1=exb[:].unsqueeze(2).to_broadcast(
                                [P, nch2, 130]),
                            op=AL.mult)
                    acc = ps.tile([P, 144], f32, space="PSUM", tag="A")
                    for s in range(nch2):
                        nc.tensor.matmul(out=acc[:, 0:tw2],
                                         lhsT=u_all[:, s * P:(s + 1) * P],
                                         rhs=gt[:, s * 256:s * 256 + tw2],
                                         start=(s == 0), stop=(s == nch2 - 1),
                                         skip_group_check=True)
                    # ----- flush -----
                    den = wk.tile([P, nh], f32, tag="den")
                    nc.vector.tensor_scalar(out=den[:],
                                            in0=acc[:, 128 + nh:128 + 2 * nh],
                                            scalar1=1e-16, scalar2=None,
                                            op0=AL.add)
                    rec = wk.tile([P, nh], f32, tag="rec")
                    recip_newton(rec[:], den[:])
                    hb = wk.tile([P, P], f32, tag="hb")
                    if l == 1:
                        for h in range(HEADS):
                            nc.vector.tensor_scalar(
                                out=hb[:, h * DH:(h + 1) * DH],
                                in0=acc[:, h * DH:(h + 1) * DH],
                                scalar1=rec[:, h:h + 1], scalar2=None,
                                op0=AL.mult)
                    else:
                        nc.vector.tensor_scalar(out=hb[:], in0=acc[:, 0:P],
                                                scalar1=rec[:], scalar2=None,
                                                op0=AL.mult)
                    if resid_res is not None:
                        nc.vector.tensor_tensor(
                            out=hb[:], in0=hb[:],
                            in1=resid_res[:, b * P:(b + 1) * P], op=AL.add)
                    else:
                        nc.vector.tensor_tensor(out=hb[:], in0=hb[:],
                                                in1=hvbn_b[1][:], op=AL.add)
                    nc.scalar.activation(out=hb[:], in_=hb[:], func=AF.Lrelu,
                                         alpha=alpha_t[:])
                    if l < 2:
                        build_table(l + 1, b, hb[:])
                    else:
                        out_head(b, hb[:])
                    if store_res is not None:
                        nc.vector.tensor_tensor(
                            out=store_res[:, b * P:(b + 1) * P], in0=hb[:],
                            in1=hvbn_b[2][:], op=AL.add)

            import os
            _bis = os.environ.get("KBISECT", "full")
            if _bis == "p0":
                zz = res.tile([P, D_OUT], f32)
                nc.vector.memset(zz[:], 0.0)
                for b in range(NBLK):
                    v = blk_valid(b)
                    nc.sync.dma_start(out=d_out[b * 128: b * 128 + v, :],
                                      in_=zz[:v, :])
            else:
                attn_layer(0, x0_res, None)
                allgather(1, "cc1")
                if _bis == "l0":
                    zz = res.tile([P, D_OUT], f32)
                    nc.vector.memset(zz[:], 0.0)
                    for b in range(NBLK):
                        v = blk_valid(b)
                        nc.sync.dma_start(out=d_out[b * 128: b * 128 + v, :],
                                          in_=zz[:v, :])
                else:
                    attn_layer(1, None, r_res)
                    allgather(2, "cc2")
                    if _bis == "l1":
                        zz = res.tile([P, D_OUT], f32)
                        nc.vector.memset(zz[:], 0.0)
                        for b in range(NBLK):
                            v = blk_valid(b)
                            nc.sync.dma_start(
                                out=d_out[b * 128: b * 128 + v, :],
                                in_=zz[:v, :])
                    else:
                        attn_layer(2, r_res, None)

    _split_multi_waits(nc, 1)
    lower_extended_insts(nc)
    return nc


class tile_pools:
    """All pools opened/closed together."""

    def __init__(self, tc):
        self.tc = tc

    def __enter__(self):
        tc = self.tc
        self.cms = [
            tc.tile_pool(name="res", bufs=1),
            tc.tile_pool(name="wk", bufs=4),
            tc.tile_pool(name="gat", bufs=4),
            tc.tile_pool(name="u", bufs=4),
            tc.tile_pool(name="ps", bufs=5, space="PSUM"),
            tc.tile_pool(name="pst", bufs=3, space="PSUM"),
        ]
        return tuple(cm.__enter__() for cm in self.cms)

    def __exit__(self, *a):
        for cm in reversed(self.cms):
            cm.__exit__(*a)
        return False


# ---------------------------------------------------------------------------
# execution: replicate bass2jax.run_bass_via_pjrt's multi-core path, but keep
# a handle on the compiled executable + inputs so a bench harness can re-time
# repeated executions without recompiling.
# ---------------------------------------------------------------------------

_LAST_RUN = None


def _run_spmd_keep(nc, in_maps, n_cores):
    global _LAST_RUN
    import jax
    from jax.experimental.shard_map import shard_map
    from jax.sharding import Mesh, PartitionSpec

    from concourse import bass2jax

    bass2jax.install_neuronx_cc_hook()
    assert nc.dbg_addr is None
    partition_name = (nc.partition_id_tensor.name
                      if nc.partition_id_tensor else None)

    in_names, out_names, out_avals, zero_outs = [], [], [], []
    for alloc in nc.m.functions[0].allocations:
        if not isinstance(alloc, mybir.MemoryLocationSet):
            continue
        name = alloc.memorylocations[0].name
        if alloc.kind == "ExternalInput":
            if name != partition_name:
                in_names.append(name)
        elif alloc.kind == "ExternalOutput":
            shape = tuple(alloc.tensor_shape)
            dtype = mybir.dt.np(alloc.dtype)
            out_names.append(name)
            out_avals.append(jax.core.ShapedArray(shape, dtype))
            zero_outs.append(np.zeros(shape, dtype))
    n_params = len(in_names)
    n_outs = len(out_avals)
    all_names = in_names + out_names
    if partition_name is not None:
        all_names = all_names + [partition_name]
    donate = tuple(range(n_params, n_params + n_outs))

    def _body(*args):
        operands = list(args)
        if partition_name is not None:
            operands.append(bass2jax.partition_id_tensor())
        outs = bass2jax._bass_exec_p.bind(
            *operands,
            out_avals=tuple(out_avals),
            in_names=tuple(all_names),
            out_names=tuple(out_names),
            lowering_input_output_aliases=(),
            sim_require_finite=True,
            sim_require_nnan=True,
            nc=nc,
        )
        return tuple(outs)

    devices = jax.devices()[:n_cores]
    assert len(devices) == n_cores
    mesh = Mesh(np.asarray(devices), ("core",))
    in_specs = (PartitionSpec("core"),) * (n_params + n_outs)
    out_specs = (PartitionSpec("core"),) * n_outs
    sharded = jax.jit(
        shard_map(_body, mesh=mesh, in_specs=in_specs, out_specs=out_specs,
                  check_rep=False),
        donate_argnums=donate, keep_unused=True,
    )
    per_core = [[np.asarray(m[name]) for name in in_names] for m in in_maps]
    concat_in = [
        np.concatenate([per_core[c][i] for c in range(n_cores)], axis=0)
        for i in range(n_params)
    ]
    concat_zeros = [
        np.zeros((n_cores * z.shape[0], *z.shape[1:]), z.dtype)
        for z in zero_outs
    ]
    out_arrs = sharded(*concat_in, *concat_zeros)
    _LAST_RUN = dict(
        sharded=sharded, concat_in=concat_in, mesh=mesh,
        zero_specs=[(z.shape, z.dtype) for z in concat_zeros],
        out_names=out_names, out_avals=out_avals, n_cores=n_cores,
    )
    return [
        {
            name: np.asarray(out_arrs[i]).reshape(n_cores, *out_avals[i].shape)[c]
            for i, name in enumerate(out_names)
        }
        for c in range(n_cores)
    ]


# ---------------------------------------------------------------------------
# public entry point
# ---------------------------------------------------------------------------

def kernel(**inputs):
    _apply_patches()
    import ml_dtypes
    bf = ml_dtypes.bfloat16

    x = np.asarray(inputs["x"], np.float32)
    edge_index = np.asarray(inputs["edge_index"], np.int32)

    SL, SH, SLt, SHt, C2, rowrel, idxa, idxb, rowloc = _preprocess(edge_index)

    W_in = np.asarray(inputs["W_in"], np.float32)
    b_in = np.asarray(inputs["b_in"], np.float32)
    W_agg = np.asarray(inputs["W_agg"], np.float32)
    b_agg = np.asarray(inputs["b_agg"], np.float32)
    sh_Wv = np.asarray(inputs["sh_Wv"], np.float32)
    sh_b = np.asarray(inputs["sh_b"], np.float32)
    sh_asrc = np.asarray(inputs["sh_asrc"], np.float32)
    sh_adst = np.asarray(inputs["sh_adst"], np.float32)
    mh_Wv = np.asarray(inputs["mh_Wv"], np.float32)
    mh_b = np.asarray(inputs["mh_b"], np.float32)
    mh_asrc = np.asarray(inputs["mh_asrc"], np.float32)
    mh_adst = np.asarray(inputs["mh_adst"], np.float32)
    bn_g = np.asarray(inputs["bn_gamma"], np.float32)
    bn_b = np.asarray(inputs["bn_beta"], np.float32)
    bn_m = np.asarray(inputs["bn_mean"], np.float32)
    bn_v = np.asarray(inputs["bn_var"], np.float32)
    W_o1 = np.asarray(inputs["W_o1"], np.float32)
    b_o1 = np.asarray(inputs["b_o1"], np.float32)
    W_o2 = np.asarray(inputs["W_o2"], np.float32)
    b_o2 = np.asarray(inputs["b_o2"], np.float32)

    bnsc = (bn_g / np.sqrt(bn_v + BN_EPS)).astype(np.float32)
    bnsh = (bn_b - bn_m * bnsc).astype(np.float32)

    # wcat0 rows pair with lhsT rows: 0:18 x | 18 ones | 32:50 nmean
    wcat0 = np.zeros((52, P), np.float32)
    wcat0[0:18] = W_in
    wcat0[18] = b_in + b_agg
    wcat0[32:50] = W_agg

    # table-build weights: cols 0:128 BN-folded values | 128:128+nh a_dst |
    # 128+nh:128+2nh zeros (ones via memset) | 136:136+nh a_src
    wv = np.zeros((3, P, 140), np.float32)
    wv[0, :, 0:P] = sh_Wv[0] * bnsc[0][None, :]
    wv[0, :, 128] = sh_adst[0]
    wv[0, :, 136] = sh_asrc[0]
    wv[1, :, 0:P] = mh_Wv.transpose(1, 0, 2).reshape(P, P) * bnsc[1][None, :]
    wv[1, :, 128:132] = mh_adst.T
    wv[1, :, 136:140] = mh_asrc.T
    wv[2, :, 0:P] = sh_Wv[1] * bnsc[2][None, :]
    wv[2, :, 128] = sh_adst[1]
    wv[2, :, 136] = sh_asrc[1]

    # value bias (BN-folded) + BN shift, applied post-softmax via residuals
    hvbn = np.zeros((3, P), np.float32)
    hvbn[0] = sh_b[0] * bnsc[0] + bnsh[0]
    hvbn[1] = mh_b.reshape(P) * bnsc[1] + bnsh[1]
    hvbn[2] = sh_b[1] * bnsc[2] + bnsh[2]

    x128 = np.zeros((N, 128), np.float32)
    x128[:, :D_IN] = x
    x128[:, D_IN] = 1.0

    xT = np.zeros((NCORES, NBLK, 20, P), np.float32)
    xb3 = np.zeros((NCORES, P, NBLK * 3), np.float32)
    for ci in range(NCORES):
        shp = np.zeros((NBLK * 128, 20), np.float32)
        shp[:NSH] = x128[ci * NSH:(ci + 1) * NSH, 0:20]
        xT[ci] = shp.reshape(NBLK, 128, 20).transpose(0, 2, 1)
        x3 = np.zeros((NBLK * 128, 3), np.float32)
        x3[:NSH] = x[ci * NSH:(ci + 1) * NSH, -3:]
        xb3[ci] = x3.reshape(NBLK, 128, 3).transpose(1, 0, 2).reshape(
            P, NBLK * 3)

    nc = _build(SL, SH, SLt, SHt, C2)

    in_maps = []
    for ci in range(NCORES):
        in_maps.append({
            "rowrel": rowrel[ci], "idxa": idxa[ci], "idxb": idxb[ci],
            "rowloc": rowloc[ci],
            "x128": x128.astype(bf), "xT": xT[ci], "xb3": xb3[ci],
            "wcat0": wcat0, "wv": wv.astype(bf), "hvbn": hvbn,
            "wo1": W_o1, "bo1": b_o1[None, :], "wo2": W_o2,
            "bo2": b_o2[None, :],
        })

    res = _run_spmd_keep(nc, in_maps, NCORES)
    out = np.concatenate([res[ci]["out"] for ci in range(NCORES)], axis=0)
    return out.astype(np.float32)
